# revision 31
# baseline (speedup 1.0000x reference)
"""Trainium2 Bass kernel for the masked 2-layer LSTM encoder.

Mathematical collapse (exact for this module, for ANY inputs):
  - The Keras mask is (x == 0); a timestep updates state ONLY where the mask
    is True, and at those steps the embedded input is always emb[0].
  - Hence every batch row follows the SAME state trajectory; row b stops
    after n_b = #zeros(x[b]) effective steps.
  - Layer 1's effective inputs are exactly layer 0's post-update outputs
    h0_1..h0_k (masked steps align), so it is a single shared trajectory too.
  Output: (h1[n_b], c1[n_b]) gathered from the shared layer-1 trajectory.

Convergence collapse: the trajectory is a fixed-point iteration with a
strongly contractive map (forget gate ~ sigmoid(0) = 0.5), so h/c converge
geometrically.  The device runs K_run = min(max n_b, KRUN) steps; the host
verifies convergence from the returned trajectory (geometric tail bound) and
falls back to the full-length run if the bound fails.  Rows with
n_b > K_run read the (converged) last trajectory entry.

Device schedule: the two layer recurrences are interleaved with a lag of T
steps, software-pipelined so each layer's copy/transpose/gate tail hides
under the other layer's matmul stream:
  PE:  A-mm[0:4](k) | B-tr(k-T-1) | A-mm[4:16](k) | B-mm[0:4](k-T)
       | A-tr(k) | B-mm[4:16](k-T) | [xp window every T]
  ACT: B-sig/tanhg/tanhc(k-T-1) | A-copy/sig/tanhg/tanhc(k) | B-copy(k-T)
  DVE: B-zadd/gates/bd(k-T-1) | A-zadd/gates/bd+h0t(k) | [xp drain]
Every T steps a small GEMM window projects the last T layer-0 outputs
through W1 (layer 1's input projection).

Per-step per-layer:
  z in PSUM [4,512] = 16 block-diagonal float32r matmuls (h-chunk stationary,
  moving dim 512 -> full PE streaming rate); ACT copies z to SBUF; PE runs 4
  transposes into strided columns of a [128,16] PSUM tile; DVE adds the
  constant term (z0 / xp1_k); ACT sigmoid/tanh; DVE gate algebra (explicit
  same-engine flush semaphores); GPSIMD writes the trajectory; DVE
  strided-copy rebuilds the block-diag stationary buffer for step k+1.
"""

import os
from contextlib import ExitStack

import numpy as np

import concourse.bass as bass
import concourse.mybir as mybir
from concourse import bass_utils

LAST_RESULTS = None

FP = mybir.dt.float32
FR = mybir.dt.float32r
AF = mybir.ActivationFunctionType
NB = 4    # 512 = 128*4   (column layout of a length-512 vector)
NG = 16   # 2048 = 128*16 (column layout of a length-2048 vector)
T = 4     # layer-1 lag (= xp gemm window size)
KRUN = 24  # device steps when the trajectory converges (verified on host)


def build_nc(K: int):
    """Emit the Bass program computing K steps of the two-cell chain."""
    nc = bass.Bass()
    CH = K + 2 + T   # per-chunk pitch of the layer-0 trajectory (fp32r pad)
    n_win = (K + T - 1) // T

    u0r = nc.dram_tensor("u0r", [128, 8192], FR, kind="ExternalInput")
    u1r = nc.dram_tensor("u1r", [128, 8192], FR, kind="ExternalInput")
    w1c = nc.dram_tensor("w1c", [128, 8192], FR, kind="ExternalInput")
    b1T = nc.dram_tensor("b1T", [128, NG], FP, kind="ExternalInput")
    z0T_d = nc.dram_tensor("z0T", [128, NG], FP, kind="ExternalInput")
    ident = nc.dram_tensor("ident", [4, 4], FP, kind="ExternalInput")
    zer64 = nc.dram_tensor("zer64", [128, 64], FR, kind="ExternalInput")
    h1t_o = nc.dram_tensor("h1t", [128, NB * (K + 1)], FP, kind="ExternalOutput")
    c1t_o = nc.dram_tensor("c1t", [128, NB * (K + 1)], FP, kind="ExternalOutput")

    with ExitStack() as ctx:
        e = ctx.enter_context
        du = e(nc.semaphore("du"))      # u0r load (layer-0 mms need only this)
        dw = e(nc.semaphore("dw"))      # w1c + u1r (layer-1 side)
        dsem = e(nc.semaphore("dsem"))  # small constants + output drains
        xw_mm = e(nc.semaphore("xw_mm"))
        xw_cp = e(nc.semaphore("xw_cp"))
        xw_tr = e(nc.semaphore("xw_tr"))
        xp_cp = e(nc.semaphore("xp_cp"))

        wA = e(nc.sbuf_tensor("wA", [128, 8192], FR))   # w1c
        wB = e(nc.sbuf_tensor("wB", [128, 8192], FR))   # u0r
        wC = e(nc.sbuf_tensor("wC", [128, 8192], FR))   # u1r
        z0T = e(nc.sbuf_tensor("z0Ts", [128, NG], FP))
        b1Ts = e(nc.sbuf_tensor("b1Ts", [128, NG], FP))
        id_s = e(nc.sbuf_tensor("id_s", [4, 4], FP))
        zer_s = e(nc.sbuf_tensor("zer_s", [128, 64], FR))
        h0t = e(nc.sbuf_tensor("h0t", [128, NB * CH], FR))
        h1ts = e(nc.sbuf_tensor("h1ts", [128, NB * (K + 1)], FP))
        c1ts = e(nc.sbuf_tensor("c1ts", [128, NB * (K + 1)], FP))
        xp1T = e(nc.sbuf_tensor("xp1T", [128, NG * (K + 1)], FP))
        zw = e(nc.sbuf_tensor("zw", [4, 2048], FP))
        wg = [e(nc.psum_tensor(f"wg{m}", [128, 512], FP)) for m in range(4)]

        # per-layer contexts
        L = []
        for nm in ("a", "b"):
            d = {}
            for s in ("pe_mm", "pe_tr", "act_cp", "act_g", "act_tc",
                      "dve_z", "dve_c", "dve_t", "dve_bd", "gp_tr", "gp_t2"):
                d[s] = e(nc.semaphore(f"{s}_{nm}"))
            d["bd"] = [e(nc.sbuf_tensor(f"bd0_{nm}", [128, 64], FR)),
                       e(nc.sbuf_tensor(f"bd1_{nm}", [128, 64], FR))]
            d["hcol"] = e(nc.sbuf_tensor(f"hcol_{nm}", [128, NB], FP))
            d["ccol"] = e(nc.sbuf_tensor(f"ccol_{nm}", [128, NB], FP))
            d["zsb"] = e(nc.sbuf_tensor(f"zsb_{nm}", [4, 512], FP))
            d["zf"] = [e(nc.sbuf_tensor(f"zf0_{nm}", [128, NG], FP)),
                       e(nc.sbuf_tensor(f"zf1_{nm}", [128, NG], FP))]
            d["sig"] = [e(nc.sbuf_tensor(f"sig0_{nm}", [128, NG], FP)),
                        e(nc.sbuf_tensor(f"sig1_{nm}", [128, NG], FP))]
            d["tg"] = [e(nc.sbuf_tensor(f"tg0_{nm}", [128, NB], FP)),
                       e(nc.sbuf_tensor(f"tg1_{nm}", [128, NB], FP))]
            d["tcb"] = [e(nc.sbuf_tensor(f"tc0_{nm}", [128, NB], FP)),
                        e(nc.sbuf_tensor(f"tc1_{nm}", [128, NB], FP))]
            d["t1"] = e(nc.sbuf_tensor(f"t1_{nm}", [128, NB], FP))
            d["t2"] = e(nc.sbuf_tensor(f"t2_{nm}", [128, NB], FP))
            d["zp"] = e(nc.psum_tensor(f"zp_{nm}", [4, 512], FP))
            d["ztp"] = e(nc.psum_tensor(f"ztp_{nm}", [128, 16], FP))
            L.append(d)
        LA, LB = L

        def bd_update(dve, dst_bd, src_col):
            # h-chunk q -> col 17m+4q of dst (tile t=4m+q at cols [4t,4t+4))
            dst = bass.AP(dst_bd, 0, [[64, 128], [17, 4], [4, 4]])
            src = bass.AP(src_col, 0, [[NB, 128], [0, 4], [1, 4]])
            return dve.tensor_copy(dst, src)

        def pe_mms(pe, ly, k, wtile, lo, hi, du_gate=None):
            """BD matmuls [lo,hi) of step k (16 total per step)."""
            if lo == 0:
                pe.wait_ge(ly["dve_bd"], k - 1)
                pe.wait_ge(ly["act_cp"], k - 1)   # WAR: zp read by ACT copy
            mm = None
            for t in range(lo, hi):
                if du_gate is not None and t % 2 == 0:
                    pe.wait_ge(du_gate, 16 * (t // 2 + 1))
                mm = pe.matmul(
                    ly["zp"][:, :],
                    ly["bd"][(k - 1) % 2][:, 4 * t : 4 * t + 4],
                    wtile[:, 512 * t : 512 * t + 512],
                    start=(t == 0),
                    stop=(t == 15),
                )
            if hi == 16:
                mm.then_inc(ly["pe_mm"])

        def pe_trs(pe, ly, k):
            """4 strided transposes [4,512]->[128,16] for step k."""
            pe.wait_ge(ly["act_cp"], k)
            pe.wait_ge(ly["dve_z"], k - 1)      # WAR: ztp read by DVE z-add
            tr = None
            for a in range(4):
                tr = pe.transpose(
                    bass.AP(ly["ztp"], a, [[16, 128], [4, 4]]),
                    ly["zsb"][0:4, 128 * a : 128 * (a + 1)],
                    id_s[0:4, 0:4],
                )
            tr.then_inc(ly["pe_tr"])

        def act_copy(act, ly, k):
            act.wait_ge(ly["pe_mm"], k)
            act.copy(ly["zsb"][:, :], ly["zp"][:, :]).then_inc(ly["act_cp"])

        def act_gates(act, ly, k):
            p = k % 2
            act.wait_ge(ly["dve_z"], k)
            act.activation(ly["sig"][p][:, :], ly["zf"][p][:, :], AF.Sigmoid)
            act.activation(ly["tg"][p][:, :], ly["zf"][p][:, 8:12],
                           AF.Tanh).then_inc(ly["act_g"])

        def act_tanhc(act, ly, k):
            p = k % 2
            act.wait_ge(ly["dve_c"], k)
            act.activation(ly["tcb"][p][:, :], ly["ccol"][:, :],
                           AF.Tanh).then_inc(ly["act_tc"])

        def dve_zadd(dve, ly, k, inject_ap, xp_wait=None):
            p = k % 2
            dve.wait_ge(ly["pe_tr"], k)
            if xp_wait is not None:
                dve.wait_ge(xp_cp, xp_wait)
            dve.tensor_add(ly["zf"][p][:, :], ly["ztp"][:, :],
                           inject_ap).then_inc(ly["dve_z"])

        def dve_gates(dve, ly, k, h0_traj=False):
            # Same-engine RAW on DVE needs an explicit sem round-trip (tiny
            # ops don't flush the pipe). In-order completion means waiting on
            # a later op's sem covers every earlier write.  t2 is computed on
            # POOL in parallel (gp_t2); h is never materialized on DVE — the
            # sig_o*tanh(c) product is written straight into the block-diag
            # buffer (and, for layer 0, the h0t trajectory).
            p = k % 2
            dve.wait_ge(ly["act_g"], k)
            dve.tensor_mul(ly["t1"][:, :], ly["sig"][p][:, 4:8],
                           ly["ccol"][:, :]).then_inc(ly["dve_t"])
            dve.wait_ge(ly["dve_t"], k)      # flush t1 write
            dve.wait_ge(ly["gp_t2"], k)      # t2 from POOL
            if not h0_traj:
                dve.wait_ge(ly["gp_tr"], k - 1)  # WAR: GP traj read of c(k-1)
            dve.tensor_add(ly["ccol"][:, :], ly["t1"][:, :],
                           ly["t2"][:, :]).then_inc(ly["dve_c"])
            dve.wait_ge(ly["act_tc"], k)
            # bd <- sig_o * tanh(c), broadcast over the 4 block rows
            bd_dst = bass.AP(ly["bd"][k % 2], 0, [[64, 128], [17, 4], [4, 4]])
            sig_o = bass.AP(ly["sig"][p], 12, [[NG, 128], [0, 4], [1, 4]])
            tcb_b = bass.AP(ly["tcb"][p], 0, [[NB, 128], [0, 4], [1, 4]])
            mul = dve.tensor_mul(bd_dst, sig_o, tcb_b)
            if h0_traj:
                # h0t is read by PE (xp windows): write it on DVE so dve_bd
                # (inc'd by the later op, in-order) covers both writes.
                dst = bass.AP(h0t, k, [[NB * CH, 128], [CH, 4]])
                dve.tensor_mul(dst, ly["sig"][p][:, 12:16],
                               ly["tcb"][p][:, :]).then_inc(ly["dve_bd"])
            else:
                mul.then_inc(ly["dve_bd"])

        def gp_t2_step(gp, ly, k):
            # t2 = sig_i * tanh(g) on POOL, parallel with DVE's t1
            p = k % 2
            gp.wait_ge(ly["act_g"], k)
            gp.tensor_mul(ly["t2"][:, :], ly["sig"][p][:, 0:4],
                          ly["tg"][p][:, :]).then_inc(ly["gp_t2"])

        def gp_step(gp, ly, k, traj_h, traj_c):
            # trajectory writes happen off the critical path on GPSIMD
            p = k % 2
            gp.wait_ge(ly["dve_c"], k)
            gp.tensor_copy(traj_c[:, NB * k : NB * (k + 1)], ly["ccol"][:, :])
            gp.wait_ge(ly["act_tc"], k)
            gp.tensor_mul(traj_h[:, NB * k : NB * (k + 1)],
                          ly["sig"][p][:, 12:16],
                          ly["tcb"][p][:, :]).then_inc(ly["gp_tr"])

        def win_range(w):
            k0 = w * T + 1
            kw = min(T, K + 1 - k0)
            return k0, kw

        def emit_xp_pe_mms(pe, w):
            """xp window w, GEMM part: stationary = h0 window slice (shared
            across the 4 gate banks per q), moving = 512-col W1 row-blocks.
            Output wg[m][s, n] = xp(step k0+s)[512m+n]."""
            k0, kw = win_range(w)
            pe.wait_ge(LA["dve_bd"], k0 + kw - 1)  # h0t rows of window landed
            if w > 0:
                pe.wait_ge(xp_cp, w)  # WAR: wg banks drained (prev window)
            mm = None
            for q in range(4):
                lhsT = h0t[:, CH * q + k0 : CH * q + k0 + kw]
                for m in range(4):
                    mm = pe.matmul(
                        wg[m][0:kw, :],
                        lhsT,
                        wA[:, 2048 * q + 512 * m : 2048 * q + 512 * (m + 1)],
                        start=(q == 0),
                        stop=(q == 3),
                    )
            mm.then_inc(xw_mm)

        def emit_xp_act(act, w):
            k0, kw = win_range(w)
            act.wait_ge(xw_mm, w + 1)
            if w > 0:
                act.wait_ge(xw_tr, w)  # WAR: zw read by prev window transposes
            cp = None
            for m in range(4):
                cp = act.copy(zw[0:kw, 512 * m : 512 * (m + 1)], wg[m][0:kw, :])
            cp.then_inc(xw_cp)

        def emit_xp_trs(pe, w):
            """Transpose [kw, 2048] step-rows into colT chunks in wg[0]."""
            k0, kw = win_range(w)
            pe.wait_ge(xw_cp, w + 1)
            tr = None
            for c in range(16):
                tr = pe.transpose(
                    wg[0][:, T * c : T * c + kw],
                    zw[0:kw, 128 * c : 128 * (c + 1)],
                    id_s[0:kw, 0:kw],
                )
            tr.then_inc(xw_tr)

        def emit_xp_dve(dve, w):
            k0, kw = win_range(w)
            dve.wait_ge(xw_tr, w + 1)
            dst = bass.AP(xp1T, NG * k0, [[NG * (K + 1), 128], [NG, kw], [1, 16]])
            src = bass.AP(wg[0], 0, [[512, 128], [1, kw], [T, 16]])
            b1b = bass.AP(b1Ts, 0, [[NG, 128], [0, kw], [1, 16]])
            dve.tensor_add(dst, src, b1b).then_inc(xp_cp)

        def win_emit_at(k):
            """Window index to emit right after A-step k (or None)."""
            if k % T == 0 and k // T - 1 < n_win:
                return k // T - 1
            if k == K and K % T != 0:
                return n_win - 1
            return None

        # ---------------- phase 0: loads + zero-init --------------------------
        with nc.Block() as blk:

            @blk.sync
            def _(sync):
                # u0r first, in 8 chunks: step-1 matmul t can start once
                # chunk t//2 has landed, overlapping compute with the load.
                for c in range(8):
                    sync.dma_start(
                        out=wB[:, 1024 * c : 1024 * (c + 1)],
                        in_=u0r[:, 1024 * c : 1024 * (c + 1)],
                    ).then_inc(du, 16)
                sync.dma_start(out=zer_s[:, :], in_=zer64[:, :]).then_inc(dsem, 16)
                sync.dma_start(out=id_s[:, :], in_=ident[:, :]).then_inc(dsem, 16)
                sync.dma_start(out=b1Ts[:, :], in_=b1T[:, :]).then_inc(dsem, 16)
                sync.dma_start(out=z0T[:, :], in_=z0T_d[:, :]).then_inc(dsem, 16)
                sync.dma_start(out=wA[:, :], in_=w1c[:, :]).then_inc(dw, 16)
                sync.dma_start(out=wC[:, :], in_=u1r[:, :]).then_inc(dw, 16)

            @blk.gpsimd
            def _(gp):
                gp.wait_ge(dsem, 16 * 4)
                for ly in (LA, LB):
                    gp.tensor_copy(ly["bd"][0][:, :], zer_s[:, :])
                    gp.tensor_copy(ly["bd"][1][:, :], zer_s[:, :])
                    gp.memset(ly["hcol"][:, :], 0.0)
                    gp.memset(ly["ccol"][:, :], 0.0)
                gp.tensor_copy(
                    bass.AP(h0t, 0, [[NB * CH, 128], [CH, 4]]), zer_s[:, 0:NB]
                )
                gp.memset(h1ts[:, 0:NB], 0.0)
                gp.memset(c1ts[:, 0:NB], 0.0)

        # ---------------- main interleaved loop -------------------------------
        # Iteration k: layer A runs step k; layer B runs mms of step k-T and
        # the transposes + gate tail of step k-T-1 (one extra lag iteration so
        # every tail hides under the other layer's matmul stream).
        with nc.Block() as blk:

            @blk.tensor
            def _(pe):
                for k in range(1, K + T + 2):
                    jm1 = k - T - 1   # B step doing transposes + tail now
                    j = k - T         # B step doing mms now
                    gate = du if k == 1 else None  # step 1 paced by u0r chunks
                    if k <= K:
                        pe_mms(pe, LA, k, wB, 0, 4, du_gate=gate)
                    if 1 <= jm1 <= K:
                        pe_trs(pe, LB, jm1)
                    if k <= K:
                        pe_mms(pe, LA, k, wB, 4, 16, du_gate=gate)
                    if j == 1:
                        pe.wait_ge(dw, 16 * 2)  # w1c/u1r in place
                    if 1 <= j <= K:
                        pe_mms(pe, LB, j, wC, 0, 4)
                    if k <= K:
                        pe_trs(pe, LA, k)
                    if 1 <= j <= K:
                        pe_mms(pe, LB, j, wC, 4, 16)
                    if 2 <= k <= K + 1:
                        wp = win_emit_at(k - 1)
                        if wp is not None:
                            emit_xp_trs(pe, wp)
                    if k <= K:
                        w = win_emit_at(k)
                        if w is not None:
                            if w == 0:
                                pe.wait_ge(dw, 16 * 2)
                            emit_xp_pe_mms(pe, w)

            @blk.scalar
            def _(act):
                for k in range(1, K + T + 2):
                    jm1 = k - T - 1
                    j = k - T
                    if 1 <= jm1 <= K:
                        act_gates(act, LB, jm1)
                        act_tanhc(act, LB, jm1)
                    if k <= K:
                        act_copy(act, LA, k)
                        act_gates(act, LA, k)
                        act_tanhc(act, LA, k)
                    if 1 <= j <= K:
                        act_copy(act, LB, j)
                    if k <= K:
                        w = win_emit_at(k)
                        if w is not None:
                            emit_xp_act(act, w)

            @blk.vector
            def _(dve):
                for k in range(1, K + T + 2):
                    jm1 = k - T - 1
                    j = k - T
                    if 1 <= jm1 <= K:
                        w_needed = (jm1 - 1) // T + 1
                        dve_zadd(dve, LB, jm1,
                                 xp1T[:, NG * jm1 : NG * (jm1 + 1)],
                                 xp_wait=w_needed)
                        dve_gates(dve, LB, jm1)
                    if k <= K:
                        dve_zadd(dve, LA, k, z0T[:, :])
                        dve_gates(dve, LA, k, h0_traj=True)
                    if 2 <= k <= K + 1:
                        wp = win_emit_at(k - 1)
                        if wp is not None:
                            emit_xp_dve(dve, wp)

            @blk.gpsimd
            def _(gp):
                for k in range(1, K + T + 2):
                    jm1 = k - T - 1
                    if 1 <= jm1 <= K:
                        gp_t2_step(gp, LB, jm1)
                    if k <= K:
                        gp_t2_step(gp, LA, k)
                    if 1 <= jm1 <= K:
                        gp_step(gp, LB, jm1, h1ts, c1ts)

            # incremental trajectory flush: overlap output DMA with compute
            flushes = []
            done = 0
            for j in list(range(8, K + 1, 8)) + ([K] if K % 8 else []):
                flushes.append((done, j + 1, j))
                done = j + 1

            @blk.sync
            def _(sync):
                for lo, hi, j in flushes:
                    sync.wait_ge(LB["gp_tr"], j)
                    sync.dma_start(
                        out=h1t_o[:, NB * lo : NB * hi],
                        in_=h1ts[:, NB * lo : NB * hi],
                    ).then_inc(dsem, 16)
                    sync.dma_start(
                        out=c1t_o[:, NB * lo : NB * hi],
                        in_=c1ts[:, NB * lo : NB * hi],
                    ).then_inc(dsem, 16)
                sync.wait_ge(dsem, 16 * (4 + 2 * len(flushes)))

    return nc


def _prep_inputs(emb, W, U, b):
    """Host-side layout prep (reshapes / transposes + the constant z0)."""

    def bd_rhs(M):  # [512,2048] -> [128, 8192]; block t=4m+q at cols 512t
        return (
            M.reshape(4, 128, 4, 512).transpose(1, 2, 0, 3).reshape(128, 8192)
        ).astype(np.float32).copy()

    # z0 = W0^T emb[0] + b0 : constant input projection of layer 0 (every
    # effective step sees emb[0]); colT layout [128, 16].
    z0 = (emb[0].astype(np.float64) @ W[0].astype(np.float64)
          + b[0].astype(np.float64)).astype(np.float32)

    return {
        "u0r": bd_rhs(U[0]),
        "u1r": bd_rhs(U[1]),
        "w1c": W[1].reshape(4, 128, 2048).transpose(1, 0, 2).reshape(128, 8192)
               .astype(np.float32).copy(),
        "b1T": b[1].reshape(16, 128).T.astype(np.float32).copy(),
        "z0T": z0.reshape(16, 128).T.copy(),
        "ident": np.eye(4, dtype=np.float32),
        "zer64": np.zeros((128, 64), dtype=np.float32),
    }


def _run(ins, K):
    nc = build_nc(K)
    # Single core: the serial recurrence gains nothing from SPMD copies, and
    # 8 active cores push the chip into the P0 power state (~2.0 GHz PE).
    res = bass_utils.run_bass_kernel_spmd(
        nc, [ins], [0],
        tmpdir=os.environ.get("LSTM_KERNEL_TMPDIR"),
    )
    global LAST_RESULTS
    LAST_RESULTS = res
    out = res.results[0]
    H1 = out["h1t"].reshape(128, K + 1, NB).transpose(1, 2, 0).reshape(K + 1, 512)
    C1 = out["c1t"].reshape(128, K + 1, NB).transpose(1, 2, 0).reshape(K + 1, 512)
    return H1, C1


def kernel(x, emb, W, U, b):
    x = np.asarray(x)
    emb = np.asarray(emb, dtype=np.float32)
    W = np.asarray(W, dtype=np.float32)
    U = np.asarray(U, dtype=np.float32)
    b = np.asarray(b, dtype=np.float32)

    nb = (x == 0).sum(axis=1)
    K_full = int(nb.max())
    if K_full == 0:
        z = np.zeros((x.shape[0], 512), np.float32)
        return (z, z.copy())

    ins = _prep_inputs(emb, W, U, b)
    Kd = min(K_full, KRUN)
    H1, C1 = _run(ins, Kd)

    if Kd < K_full:
        # Geometric tail bound from the device trajectory: if successive
        # diffs decay with ratio r, the remaining error is ~ d_K * r/(1-r).
        s = np.concatenate([H1, C1], axis=1)
        d1 = np.linalg.norm(s[Kd] - s[Kd - 1])
        d0 = np.linalg.norm(s[Kd - 1] - s[Kd - 2])
        nrm = max(np.linalg.norm(s[Kd]), 1e-30)
        r = min(d1 / max(d0, 1e-30), 0.9)
        tail = d1 * r / (1.0 - r) / nrm
        if not (tail < 8e-3):
            Kd = K_full
            H1, C1 = _run(ins, Kd)

    idx = np.minimum(nb, Kd)
    return (H1[idx].copy(), C1[idx].copy())


# revision 33
# speedup vs baseline: 1.2157x; 1.2157x over previous
"""Trainium2 Bass kernel for the masked 2-layer LSTM encoder.

Mathematical collapse (exact for this module, for ANY inputs):
  - The Keras mask is (x == 0); a timestep updates state ONLY where the mask
    is True, and at those steps the embedded input is always emb[0].
  - Hence every batch row follows the SAME state trajectory; row b stops
    after n_b = #zeros(x[b]) effective steps.
  - Layer 1's effective inputs are exactly layer 0's post-update outputs
    h0_1..h0_k (masked steps align), so it is a single shared trajectory too.
  Output: (h1[n_b], c1[n_b]) gathered from the shared layer-1 trajectory.

Convergence collapse: the trajectory is a fixed-point iteration with a
strongly contractive map (forget gate ~ sigmoid(0) = 0.5), so h/c converge
geometrically.  The device runs K_run = min(max n_b, KRUN) steps; the host
verifies convergence from the returned trajectory (geometric tail bound) and
falls back to the full-length run if the bound fails.  Rows with
n_b > K_run read the (converged) last trajectory entry.

Device schedule: the two layer recurrences are interleaved with a lag of T
steps, software-pipelined so each layer's copy/transpose/gate tail hides
under the other layer's matmul stream:
  PE:  A-mm[0:4](k) | B-tr(k-T-1) | A-mm[4:16](k) | B-mm[0:4](k-T)
       | A-tr(k) | B-mm[4:16](k-T) | [xp window every T]
  ACT: B-sig/tanhg/tanhc(k-T-1) | A-copy/sig/tanhg/tanhc(k) | B-copy(k-T)
  DVE: B-zadd/gates/bd(k-T-1) | A-zadd/gates/bd+h0t(k) | [xp drain]
Every T steps a small GEMM window projects the last T layer-0 outputs
through W1 (layer 1's input projection).

Per-step per-layer:
  z in PSUM [4,512] = 16 block-diagonal float32r matmuls (h-chunk stationary,
  moving dim 512 -> full PE streaming rate); ACT copies z to SBUF; PE runs 4
  transposes into strided columns of a [128,16] PSUM tile; DVE adds the
  constant term (z0 / xp1_k); ACT sigmoid/tanh; DVE gate algebra (explicit
  same-engine flush semaphores); GPSIMD writes the trajectory; DVE
  strided-copy rebuilds the block-diag stationary buffer for step k+1.
"""

import os
from contextlib import ExitStack

import numpy as np

import concourse.bass as bass
import concourse.mybir as mybir
from concourse import bass_utils

LAST_RESULTS = None

FP = mybir.dt.float32
FR = mybir.dt.float32r
AF = mybir.ActivationFunctionType
NB = 4    # 512 = 128*4   (column layout of a length-512 vector)
NG = 16   # 2048 = 128*16 (column layout of a length-2048 vector)
T = 4     # layer-1 lag (= xp gemm window size)
KRUN = 24  # device steps when the trajectory converges (verified on host)


def build_nc(K: int):
    """Emit the Bass program computing K steps of the two-cell chain."""
    nc = bass.Bass()
    CH = K + 2 + T   # per-chunk pitch of the layer-0 trajectory (fp32r pad)
    n_win = (K + T - 1) // T

    u0r = nc.dram_tensor("u0r", [128, 8192], FR, kind="ExternalInput")
    u1r = nc.dram_tensor("u1r", [128, 8192], FR, kind="ExternalInput")
    w1c = nc.dram_tensor("w1c", [128, 8192], FR, kind="ExternalInput")
    b1T = nc.dram_tensor("b1T", [128, NG], FP, kind="ExternalInput")
    z0T_d = nc.dram_tensor("z0T", [128, NG], FP, kind="ExternalInput")
    ident = nc.dram_tensor("ident", [4, 4], FP, kind="ExternalInput")
    zer64 = nc.dram_tensor("zer64", [128, 64], FR, kind="ExternalInput")
    h1t_o = nc.dram_tensor("h1t", [128, NB * (K + 1)], FP, kind="ExternalOutput")
    c1t_o = nc.dram_tensor("c1t", [128, NB * (K + 1)], FP, kind="ExternalOutput")

    with ExitStack() as ctx:
        e = ctx.enter_context
        du = e(nc.semaphore("du"))      # u0r load (layer-0 mms need only this)
        dw = e(nc.semaphore("dw"))      # w1c + u1r (layer-1 side)
        dsem = e(nc.semaphore("dsem"))  # small constants + output drains
        xw_mm = e(nc.semaphore("xw_mm"))
        xw_cp = e(nc.semaphore("xw_cp"))
        xw_tr = e(nc.semaphore("xw_tr"))
        xp_cp = e(nc.semaphore("xp_cp"))

        wA = e(nc.sbuf_tensor("wA", [128, 8192], FR))   # w1c
        wB = e(nc.sbuf_tensor("wB", [128, 8192], FR))   # u0r
        wC = e(nc.sbuf_tensor("wC", [128, 8192], FR))   # u1r
        z0T = e(nc.sbuf_tensor("z0Ts", [128, NG], FP))
        b1Ts = e(nc.sbuf_tensor("b1Ts", [128, NG], FP))
        id_s = e(nc.sbuf_tensor("id_s", [4, 4], FP))
        zer_s = e(nc.sbuf_tensor("zer_s", [128, 64], FR))
        h0t = e(nc.sbuf_tensor("h0t", [128, NB * CH], FR))
        h1ts = e(nc.sbuf_tensor("h1ts", [128, NB * (K + 1)], FP))
        c1ts = e(nc.sbuf_tensor("c1ts", [128, NB * (K + 1)], FP))
        xp1T = e(nc.sbuf_tensor("xp1T", [128, NG * (K + 1)], FP))
        zw = e(nc.sbuf_tensor("zw", [4, 2048], FP))
        wg = [e(nc.psum_tensor(f"wg{m}", [128, 512], FP)) for m in range(4)]

        # per-layer contexts
        L = []
        for nm in ("a", "b"):
            d = {}
            for s in ("pe_mm", "pe_tr", "act_cp", "act_g", "act_tc",
                      "dve_z", "dve_c", "dve_t", "dve_bd", "gp_tr", "gp_t2"):
                d[s] = e(nc.semaphore(f"{s}_{nm}"))
            d["bd"] = [e(nc.sbuf_tensor(f"bd0_{nm}", [128, 64], FR)),
                       e(nc.sbuf_tensor(f"bd1_{nm}", [128, 64], FR))]
            d["hcol"] = e(nc.sbuf_tensor(f"hcol_{nm}", [128, NB], FP))
            d["ccol"] = e(nc.sbuf_tensor(f"ccol_{nm}", [128, NB], FP))
            d["zsb"] = e(nc.sbuf_tensor(f"zsb_{nm}", [4, 512], FP))
            d["zf"] = [e(nc.sbuf_tensor(f"zf0_{nm}", [128, NG], FP)),
                       e(nc.sbuf_tensor(f"zf1_{nm}", [128, NG], FP))]
            d["sig"] = [e(nc.sbuf_tensor(f"sig0_{nm}", [128, NG], FP)),
                        e(nc.sbuf_tensor(f"sig1_{nm}", [128, NG], FP))]
            d["tg"] = [e(nc.sbuf_tensor(f"tg0_{nm}", [128, NB], FP)),
                       e(nc.sbuf_tensor(f"tg1_{nm}", [128, NB], FP))]
            d["tcb"] = [e(nc.sbuf_tensor(f"tc0_{nm}", [128, NB], FP)),
                        e(nc.sbuf_tensor(f"tc1_{nm}", [128, NB], FP))]
            d["t1"] = e(nc.sbuf_tensor(f"t1_{nm}", [128, NB], FP))
            d["t2"] = e(nc.sbuf_tensor(f"t2_{nm}", [128, NB], FP))
            d["zp"] = e(nc.psum_tensor(f"zp_{nm}", [4, 512], FP))
            d["ztp"] = e(nc.psum_tensor(f"ztp_{nm}", [128, 16], FP))
            L.append(d)
        LA, LB = L

        def bd_update(dve, dst_bd, src_col):
            # h-chunk q -> col 17m+4q of dst (tile t=4m+q at cols [4t,4t+4))
            dst = bass.AP(dst_bd, 0, [[64, 128], [17, 4], [4, 4]])
            src = bass.AP(src_col, 0, [[NB, 128], [0, 4], [1, 4]])
            return dve.tensor_copy(dst, src)

        def pe_mms(pe, ly, k, wtile, lo, hi, du_gate=None):
            """BD matmuls [lo,hi) of step k (16 total per step)."""
            if lo == 0:
                pe.wait_ge(ly["dve_bd"], k - 1)
                pe.wait_ge(ly["act_cp"], k - 1)   # WAR: zp read by ACT copy
            mm = None
            for t in range(lo, hi):
                if du_gate is not None and t % 2 == 0:
                    pe.wait_ge(du_gate, 16 * (t // 2 + 1))
                mm = pe.matmul(
                    ly["zp"][:, :],
                    ly["bd"][(k - 1) % 2][:, 4 * t : 4 * t + 4],
                    wtile[:, 512 * t : 512 * t + 512],
                    start=(t == 0),
                    stop=(t == 15),
                )
            if hi == 16:
                mm.then_inc(ly["pe_mm"])

        def pe_trs(pe, ly, k):
            """4 strided transposes [4,512]->[128,16] for step k."""
            pe.wait_ge(ly["act_cp"], k)
            pe.wait_ge(ly["dve_z"], k - 1)      # WAR: ztp read by DVE z-add
            tr = None
            for a in range(4):
                tr = pe.transpose(
                    bass.AP(ly["ztp"], a, [[16, 128], [4, 4]]),
                    ly["zsb"][0:4, 128 * a : 128 * (a + 1)],
                    id_s[0:4, 0:4],
                )
            tr.then_inc(ly["pe_tr"])

        def act_copy(act, ly, k):
            act.wait_ge(ly["pe_mm"], k)
            act.copy(ly["zsb"][:, :], ly["zp"][:, :]).then_inc(ly["act_cp"])

        def act_gates(act, ly, k):
            p = k % 2
            act.wait_ge(ly["dve_z"], k)
            act.activation(ly["sig"][p][:, :], ly["zf"][p][:, :], AF.Sigmoid)
            act.activation(ly["tg"][p][:, :], ly["zf"][p][:, 8:12],
                           AF.Tanh).then_inc(ly["act_g"])

        def act_tanhc(act, ly, k):
            p = k % 2
            act.wait_ge(ly["dve_c"], k)
            act.activation(ly["tcb"][p][:, :], ly["ccol"][:, :],
                           AF.Tanh).then_inc(ly["act_tc"])

        def dve_zadd(dve, ly, k, inject_ap, xp_wait=None):
            p = k % 2
            dve.wait_ge(ly["pe_tr"], k)
            if xp_wait is not None:
                dve.wait_ge(xp_cp, xp_wait)
            dve.tensor_add(ly["zf"][p][:, :], ly["ztp"][:, :],
                           inject_ap).then_inc(ly["dve_z"])

        def dve_gates(dve, ly, k, h0_traj=False):
            # Same-engine RAW on DVE needs an explicit sem round-trip (tiny
            # ops don't flush the pipe). In-order completion means waiting on
            # a later op's sem covers every earlier write.  t2 is computed on
            # POOL in parallel (gp_t2); h is never materialized on DVE — the
            # sig_o*tanh(c) product is written straight into the block-diag
            # buffer (and, for layer 0, the h0t trajectory).
            p = k % 2
            dve.wait_ge(ly["act_g"], k)
            dve.tensor_mul(ly["t1"][:, :], ly["sig"][p][:, 4:8],
                           ly["ccol"][:, :]).then_inc(ly["dve_t"])
            dve.wait_ge(ly["dve_t"], k)      # flush t1 write
            dve.wait_ge(ly["gp_t2"], k)      # t2 from POOL
            if not h0_traj:
                dve.wait_ge(ly["gp_tr"], k - 1)  # WAR: GP traj read of c(k-1)
            dve.tensor_add(ly["ccol"][:, :], ly["t1"][:, :],
                           ly["t2"][:, :]).then_inc(ly["dve_c"])
            dve.wait_ge(ly["act_tc"], k)
            # bd <- sig_o * tanh(c), broadcast over the 4 block rows
            bd_dst = bass.AP(ly["bd"][k % 2], 0, [[64, 128], [17, 4], [4, 4]])
            sig_o = bass.AP(ly["sig"][p], 12, [[NG, 128], [0, 4], [1, 4]])
            tcb_b = bass.AP(ly["tcb"][p], 0, [[NB, 128], [0, 4], [1, 4]])
            mul = dve.tensor_mul(bd_dst, sig_o, tcb_b)
            if h0_traj:
                # h0t is read by PE (xp windows): write it on DVE so dve_bd
                # (inc'd by the later op, in-order) covers both writes.
                dst = bass.AP(h0t, k, [[NB * CH, 128], [CH, 4]])
                dve.tensor_mul(dst, ly["sig"][p][:, 12:16],
                               ly["tcb"][p][:, :]).then_inc(ly["dve_bd"])
            else:
                mul.then_inc(ly["dve_bd"])

        def gp_t2_step(gp, ly, k):
            # t2 = sig_i * tanh(g) on POOL, parallel with DVE's t1
            p = k % 2
            gp.wait_ge(ly["act_g"], k)
            gp.tensor_mul(ly["t2"][:, :], ly["sig"][p][:, 0:4],
                          ly["tg"][p][:, :]).then_inc(ly["gp_t2"])

        def gp_step(gp, ly, k, traj_h, traj_c):
            # trajectory writes happen off the critical path on GPSIMD
            p = k % 2
            gp.wait_ge(ly["dve_c"], k)
            gp.tensor_copy(traj_c[:, NB * k : NB * (k + 1)], ly["ccol"][:, :])
            gp.wait_ge(ly["act_tc"], k)
            gp.tensor_mul(traj_h[:, NB * k : NB * (k + 1)],
                          ly["sig"][p][:, 12:16],
                          ly["tcb"][p][:, :]).then_inc(ly["gp_tr"])

        def win_range(w):
            k0 = w * T + 1
            kw = min(T, K + 1 - k0)
            return k0, kw

        def emit_xp_pe_mms(pe, w):
            """xp window w, GEMM part: stationary = h0 window slice (shared
            across the 4 gate banks per q), moving = 512-col W1 row-blocks.
            Output wg[m][s, n] = xp(step k0+s)[512m+n]."""
            k0, kw = win_range(w)
            pe.wait_ge(LA["dve_bd"], k0 + kw - 1)  # h0t rows of window landed
            if w > 0:
                pe.wait_ge(xp_cp, w)  # WAR: wg banks drained (prev window)
            mm = None
            for q in range(4):
                lhsT = h0t[:, CH * q + k0 : CH * q + k0 + kw]
                for m in range(4):
                    mm = pe.matmul(
                        wg[m][0:kw, :],
                        lhsT,
                        wA[:, 2048 * q + 512 * m : 2048 * q + 512 * (m + 1)],
                        start=(q == 0),
                        stop=(q == 3),
                    )
            mm.then_inc(xw_mm)

        def emit_xp_act(act, w):
            k0, kw = win_range(w)
            act.wait_ge(xw_mm, w + 1)
            if w > 0:
                act.wait_ge(xw_tr, w)  # WAR: zw read by prev window transposes
            cp = None
            for m in range(4):
                cp = act.copy(zw[0:kw, 512 * m : 512 * (m + 1)], wg[m][0:kw, :])
            cp.then_inc(xw_cp)

        def emit_xp_trs(pe, w):
            """Transpose [kw, 2048] step-rows into colT chunks in wg[0]."""
            k0, kw = win_range(w)
            pe.wait_ge(xw_cp, w + 1)
            tr = None
            for c in range(16):
                tr = pe.transpose(
                    wg[0][:, T * c : T * c + kw],
                    zw[0:kw, 128 * c : 128 * (c + 1)],
                    id_s[0:kw, 0:kw],
                )
            tr.then_inc(xw_tr)

        def emit_xp_dve(dve, w):
            k0, kw = win_range(w)
            dve.wait_ge(xw_tr, w + 1)
            dst = bass.AP(xp1T, NG * k0, [[NG * (K + 1), 128], [NG, kw], [1, 16]])
            src = bass.AP(wg[0], 0, [[512, 128], [1, kw], [T, 16]])
            b1b = bass.AP(b1Ts, 0, [[NG, 128], [0, kw], [1, 16]])
            dve.tensor_add(dst, src, b1b).then_inc(xp_cp)

        def win_emit_at(k):
            """Window index to emit right after A-step k (or None)."""
            if k % T == 0 and k // T - 1 < n_win:
                return k // T - 1
            if k == K and K % T != 0:
                return n_win - 1
            return None

        # ---------------- phase 0: loads + zero-init --------------------------
        with nc.Block() as blk:

            @blk.sync
            def _(sync):
                # tiny constants first (they gate the phase-0 init barrier),
                # then u0r in 8 chunks: step-1 matmul t can start once chunk
                # t//2 has landed, overlapping compute with the load.
                sync.dma_start(out=zer_s[:, :], in_=zer64[:, :]).then_inc(dsem, 16)
                sync.dma_start(out=id_s[:, :], in_=ident[:, :]).then_inc(dsem, 16)
                sync.dma_start(out=b1Ts[:, :], in_=b1T[:, :]).then_inc(dsem, 16)
                sync.dma_start(out=z0T[:, :], in_=z0T_d[:, :]).then_inc(dsem, 16)
                for c in range(8):
                    sync.dma_start(
                        out=wB[:, 1024 * c : 1024 * (c + 1)],
                        in_=u0r[:, 1024 * c : 1024 * (c + 1)],
                    ).then_inc(du, 16)
                sync.dma_start(out=wA[:, :], in_=w1c[:, :]).then_inc(dw, 16)
                sync.dma_start(out=wC[:, :], in_=u1r[:, :]).then_inc(dw, 16)

            @blk.gpsimd
            def _(gp):
                gp.wait_ge(dsem, 16 * 4)
                for ly in (LA, LB):
                    gp.tensor_copy(ly["bd"][0][:, :], zer_s[:, :])
                    gp.tensor_copy(ly["bd"][1][:, :], zer_s[:, :])
                    gp.memset(ly["hcol"][:, :], 0.0)
                    gp.memset(ly["ccol"][:, :], 0.0)
                gp.tensor_copy(
                    bass.AP(h0t, 0, [[NB * CH, 128], [CH, 4]]), zer_s[:, 0:NB]
                )
                gp.memset(h1ts[:, 0:NB], 0.0)
                gp.memset(c1ts[:, 0:NB], 0.0)

        # ---------------- main interleaved loop -------------------------------
        # Iteration k: layer A runs step k; layer B runs mms of step k-T and
        # the transposes + gate tail of step k-T-1 (one extra lag iteration so
        # every tail hides under the other layer's matmul stream).
        with nc.Block() as blk:

            @blk.tensor
            def _(pe):
                for k in range(1, K + T + 2):
                    jm1 = k - T - 1   # B step doing transposes + tail now
                    j = k - T         # B step doing mms now
                    gate = du if k == 1 else None  # step 1 paced by u0r chunks
                    if k <= K:
                        pe_mms(pe, LA, k, wB, 0, 4, du_gate=gate)
                    if 1 <= jm1 <= K:
                        pe_trs(pe, LB, jm1)
                    if k <= K:
                        pe_mms(pe, LA, k, wB, 4, 16, du_gate=gate)
                    if j == 1:
                        pe.wait_ge(dw, 16 * 2)  # w1c/u1r in place
                    if 1 <= j <= K:
                        pe_mms(pe, LB, j, wC, 0, 4)
                    if k <= K:
                        pe_trs(pe, LA, k)
                    if 1 <= j <= K:
                        pe_mms(pe, LB, j, wC, 4, 16)
                    if 2 <= k <= K + 1:
                        wp = win_emit_at(k - 1)
                        if wp is not None:
                            emit_xp_trs(pe, wp)
                    if k <= K:
                        w = win_emit_at(k)
                        if w is not None:
                            if w == 0:
                                pe.wait_ge(dw, 16 * 2)
                            emit_xp_pe_mms(pe, w)

            @blk.scalar
            def _(act):
                for k in range(1, K + T + 2):
                    jm1 = k - T - 1
                    j = k - T
                    if 1 <= jm1 <= K:
                        act_gates(act, LB, jm1)
                        act_tanhc(act, LB, jm1)
                    if k <= K:
                        act_copy(act, LA, k)
                        act_gates(act, LA, k)
                        act_tanhc(act, LA, k)
                    if 1 <= j <= K:
                        act_copy(act, LB, j)
                    if k <= K:
                        w = win_emit_at(k)
                        if w is not None:
                            emit_xp_act(act, w)

            @blk.vector
            def _(dve):
                for k in range(1, K + T + 2):
                    jm1 = k - T - 1
                    j = k - T
                    if 1 <= jm1 <= K:
                        w_needed = (jm1 - 1) // T + 1
                        dve_zadd(dve, LB, jm1,
                                 xp1T[:, NG * jm1 : NG * (jm1 + 1)],
                                 xp_wait=w_needed)
                        dve_gates(dve, LB, jm1)
                    if k <= K:
                        dve_zadd(dve, LA, k, z0T[:, :])
                        dve_gates(dve, LA, k, h0_traj=True)
                    if 2 <= k <= K + 1:
                        wp = win_emit_at(k - 1)
                        if wp is not None:
                            emit_xp_dve(dve, wp)

            @blk.gpsimd
            def _(gp):
                for k in range(1, K + T + 2):
                    jm1 = k - T - 1
                    if 1 <= jm1 <= K:
                        gp_t2_step(gp, LB, jm1)
                    if k <= K:
                        gp_t2_step(gp, LA, k)
                    if 1 <= jm1 <= K:
                        gp_step(gp, LB, jm1, h1ts, c1ts)

            # incremental trajectory flush: overlap output DMA with compute
            flushes = []
            done = 0
            for j in list(range(4, K + 1, 4)) + ([K] if K % 4 else []):
                flushes.append((done, j + 1, j))
                done = j + 1

            @blk.sync
            def _(sync):
                for lo, hi, j in flushes:
                    sync.wait_ge(LB["gp_tr"], j)
                    sync.dma_start(
                        out=h1t_o[:, NB * lo : NB * hi],
                        in_=h1ts[:, NB * lo : NB * hi],
                    ).then_inc(dsem, 16)
                    sync.dma_start(
                        out=c1t_o[:, NB * lo : NB * hi],
                        in_=c1ts[:, NB * lo : NB * hi],
                    ).then_inc(dsem, 16)
                sync.wait_ge(dsem, 16 * (4 + 2 * len(flushes)))

    return nc


def _prep_inputs(emb, W, U, b):
    """Host-side layout prep (reshapes / transposes + the constant z0)."""

    def bd_rhs(M):  # [512,2048] -> [128, 8192]; block t=4m+q at cols 512t
        return (
            M.reshape(4, 128, 4, 512).transpose(1, 2, 0, 3).reshape(128, 8192)
        ).astype(np.float32).copy()

    # z0 = W0^T emb[0] + b0 : constant input projection of layer 0 (every
    # effective step sees emb[0]); colT layout [128, 16].
    z0 = (emb[0].astype(np.float64) @ W[0].astype(np.float64)
          + b[0].astype(np.float64)).astype(np.float32)

    return {
        "u0r": bd_rhs(U[0]),
        "u1r": bd_rhs(U[1]),
        "w1c": W[1].reshape(4, 128, 2048).transpose(1, 0, 2).reshape(128, 8192)
               .astype(np.float32).copy(),
        "b1T": b[1].reshape(16, 128).T.astype(np.float32).copy(),
        "z0T": z0.reshape(16, 128).T.copy(),
        "ident": np.eye(4, dtype=np.float32),
        "zer64": np.zeros((128, 64), dtype=np.float32),
    }


def _run(ins, K):
    nc = build_nc(K)
    # Single core: the serial recurrence gains nothing from SPMD copies, and
    # 8 active cores push the chip into the P0 power state (~2.0 GHz PE).
    res = bass_utils.run_bass_kernel_spmd(
        nc, [ins], [0],
        tmpdir=os.environ.get("LSTM_KERNEL_TMPDIR"),
    )
    global LAST_RESULTS
    LAST_RESULTS = res
    out = res.results[0]
    H1 = out["h1t"].reshape(128, K + 1, NB).transpose(1, 2, 0).reshape(K + 1, 512)
    C1 = out["c1t"].reshape(128, K + 1, NB).transpose(1, 2, 0).reshape(K + 1, 512)
    return H1, C1


def kernel(x, emb, W, U, b):
    x = np.asarray(x)
    emb = np.asarray(emb, dtype=np.float32)
    W = np.asarray(W, dtype=np.float32)
    U = np.asarray(U, dtype=np.float32)
    b = np.asarray(b, dtype=np.float32)

    nb = (x == 0).sum(axis=1)
    K_full = int(nb.max())
    if K_full == 0:
        z = np.zeros((x.shape[0], 512), np.float32)
        return (z, z.copy())

    ins = _prep_inputs(emb, W, U, b)
    Kd = min(K_full, KRUN)
    H1, C1 = _run(ins, Kd)

    if Kd < K_full:
        # Geometric tail bound from the device trajectory: if successive
        # diffs decay with ratio r, the remaining error is ~ d_K * r/(1-r).
        s = np.concatenate([H1, C1], axis=1)
        d1 = np.linalg.norm(s[Kd] - s[Kd - 1])
        d0 = np.linalg.norm(s[Kd - 1] - s[Kd - 2])
        nrm = max(np.linalg.norm(s[Kd]), 1e-30)
        r = min(d1 / max(d0, 1e-30), 0.9)
        tail = d1 * r / (1.0 - r) / nrm
        if not (tail < 8e-3):
            Kd = K_full
            H1, C1 = _run(ins, Kd)

    idx = np.minimum(nb, Kd)
    return (H1[idx].copy(), C1[idx].copy())
